# revision 17
# baseline (speedup 1.0000x reference)
"""Trainium2 Bass kernel for the BsPINN Helmholtz loss — fp8 DoubleRow edition.

Math identical to v1 (forward-Laplacian with streams v, gx, gy, t=m1+q), but:
  - all hidden matmuls use fp8e4 inputs with MatmulPerfMode.DoubleRow, pairing
    k-chunks (dense layers), (W,0) half-pairs (block-diagonal single-chunk
    layers), and (m1,q) stream pairs (the t-stream add is absorbed by the pair)
  - cos(z) comes from 1 - sin(z)^2/2 ... exactly: ct' = -v^2/2, and the (+1)
    is folded into the scalar_tensor_tensor ops (ct'+1)*x
  - f/bias adds ride float32r K=1 matmuls (1 cycle/row at free>=256)
  - elementwise split by ISA constraints: sin/Square of PSUM on ACT (only
    engine with single-read PSUM math), PSUM multiplies on DVE (Pool cannot
    access PSUM; no engine may read PSUM twice in one instruction), and
    SBUF-only ops (q8, r2) on Pool; boundary tiles are woven through the
    domain schedule in per-layer steps to fill engine gaps

Sharding: data-parallel, 8 cores x (8192 domain + 2048 boundary) points.
"""

import numpy as np
import ml_dtypes

import concourse.bass as bass
import concourse.bacc as bacc_mod
import concourse.mybir as mybir
import concourse.tile as tile
from concourse.bass_utils import run_bass_kernel_spmd

bf16 = ml_dtypes.bfloat16
f8 = ml_dtypes.float8_e4m3
FP32 = mybir.dt.float32
F32R = mybir.dt.float32r
BF16 = mybir.dt.bfloat16
FP8 = mybir.dt.float8e4
AF = mybir.ActivationFunctionType
ALU = mybir.AluOpType
DR = mybir.MatmulPerfMode.DoubleRow

NCORES = 8
ND, NB = 65536, 16384
TDOM, TBND = ND // NCORES, NB // NCORES  # 8192, 2048
T = 512
NTD, NTB = TDOM // T, TBND // T          # 16, 4
K0 = 8.0
K0SQ = K0 * K0
PI_2 = float(np.pi / 2)

# k-chunk sets per (hidden layer, output m-chunk) from the block-diagonal masks
KSETS = {
    1: [[0, 1, 2, 3]] * 4,
    2: [[0, 1], [0, 1], [2, 3], [2, 3]],
    3: [[0], [1], [2], [3]],
    4: [[0], [1], [2], [3]],
}
# k-pair indices (pair j covers chunks 2j, 2j+1) for the z/gx/gy streams
KPAIRS = {
    1: [[0, 1]] * 4,
    2: [[0], [0], [1], [1]],
    3: [[0], [0], [1], [1]],   # half-pair via wz (slot m%2 live)
    4: [[0], [0], [1], [1]],
}
# index into the wt (m1,q)-pair weight tensor: (l, m, ki) -> row
WT_IDX = {}
_row = 0
for _l in (2, 3, 4):
    for _m in range(4):
        for _ki, _k in enumerate(KSETS[_l][_m]):
            WT_IDX[(_l, _m, _ki)] = _row
            _row += 1
NWT = _row  # 16


# engine assignment config: per hidden layer l=1..4, engines for each op.
# 'A' = ACT (scalar), 'D' = DVE (vector), 'P' = Pool (gpsimd)
# ct: cos carrier ('A' = Sin(z+pi/2) direct cos bf16; 'D'/'P' = STT 1-v8^2/2)
# Constraints: Square/sin of PSUM: ACT only (single PSUM read per inst,
# Pool cannot access PSUM). g8/m1 (PSUM muls): DVE only. SBUF-only ops
# (ct-from-v8, r2, q8) can go to Pool.
DEFAULT_CFG = {
    "ct": {1: "D", 2: "D", 3: "D", 4: "D"},
    "sq": {1: "A", 2: "A", 3: "A", 4: "A"},
    "g8": {1: "D", 2: "D", 3: "D"},
    "m1": {1: "D", 2: "D", 3: "D", 4: "D"},
    "q8": {1: "P", 2: "P", 3: "P", 4: "P"},
    "r2": {1: "D", 2: "P", 3: "P", 4: "P"},
}


def build_nc(ntd=NTD, ntb=NTB, cfg=None):
    from contextlib import ExitStack
    cfg = cfg or DEFAULT_CFG

    td, tb = ntd * T, ntb * T
    nc = bacc_mod.Bacc("TRN2", target_bir_lowering=False)

    xa_d = nc.dram_tensor("xa", [2, td], BF16, kind="ExternalInput")
    xb_d = nc.dram_tensor("xb", [2, tb], BF16, kind="ExternalInput")
    fb_d = nc.dram_tensor("fb", [1, td], F32R, kind="ExternalInput")
    bb_d = nc.dram_tensor("bb", [1, tb], F32R, kind="ExternalInput")
    w0_d = nc.dram_tensor("w0", [2, 512], BF16, kind="ExternalInput")
    w_d = {
        l: nc.dram_tensor(f"w{l}", [128, 4, 512], FP8, kind="ExternalInput")
        for l in (1, 2)
    }
    wf_d = {
        s: nc.dram_tensor(f"w1{s}", [128, 4, 512], FP8, kind="ExternalInput")
        for s in ("x", "y", "q")
    }
    wz_d = {
        l: nc.dram_tensor(f"wz{l}", [128, 4, 2, 128], FP8, kind="ExternalInput")
        for l in (3, 4)
    }
    wt_d = nc.dram_tensor("wt", [128, NWT, 2, 128], FP8, kind="ExternalInput")
    w5t_d = nc.dram_tensor("w5t", [128, 4, 2, 1], FP8, kind="ExternalInput")
    w5v_d = nc.dram_tensor("w5v", [128, 4, 1], FP8, kind="ExternalInput")
    w5b_d = nc.dram_tensor("w5b", [128, 4, 1], FP8, kind="ExternalInput")
    one_d = nc.dram_tensor("onec", [1, 1], F32R, kind="ExternalInput")
    out_d = nc.dram_tensor("out", [1, 32], FP32, kind="ExternalOutput")

    with tile.TileContext(nc) as tc, ExitStack() as ctx:
        singles = ctx.enter_context(tc.tile_pool(name="singles", bufs=1))
        acts = ctx.enter_context(tc.tile_pool(name="acts", bufs=6))
        ew = ctx.enter_context(tc.tile_pool(name="ew", bufs=6))
        ppz = ctx.enter_context(tc.tile_pool(name="ppz", bufs=2, space="PSUM"))
        ppx = ctx.enter_context(tc.tile_pool(name="ppx", bufs=2, space="PSUM"))
        ppt = ctx.enter_context(tc.tile_pool(name="ppt", bufs=2, space="PSUM"))

        def ld(name, shape, dt, src):
            t_ = singles.tile(shape, dt, name=name, tag=name)
            nc.sync.dma_start(out=t_, in_=src[:])
            return t_

        w0_sb = ld("w0_sb", [2, 512], BF16, w0_d)
        xa_sb = ld("xa_sb", [2, td], BF16, xa_d)
        w_sb = {l: ld(f"w{l}_sb", [128, 4, 512], FP8, w_d[l]) for l in (1, 2)}
        wf_sb = {s: ld(f"w1{s}_sb", [128, 4, 512], FP8, wf_d[s]) for s in ("x", "y", "q")}
        wz_sb = {l: ld(f"wz{l}_sb", [128, 4, 2, 128], FP8, wz_d[l]) for l in (3, 4)}
        wt_sb = ld("wt_sb", [128, NWT, 2, 128], FP8, wt_d)
        xb_sb = ld("xb_sb", [2, tb], BF16, xb_d)
        fb_sb = ld("fb_sb", [1, td], F32R, fb_d)
        bb_sb = ld("bb_sb", [1, tb], F32R, bb_d)
        w5t_sb = ld("w5t_sb", [128, 4, 2, 1], FP8, w5t_d)
        w5v_sb = ld("w5v_sb", [128, 4, 1], FP8, w5v_d)
        w5b_sb = ld("w5b_sb", [128, 4, 1], FP8, w5b_d)

        one_sb = ld("one_sb", [1, 1], F32R, one_d)
        out_sb = singles.tile([1, 32], FP32, name="out_sb")
        nc.vector.memset(out_sb, 0.0)

        pi2_sb = singles.tile([128, 1], FP32, name="pi2_sb")
        nc.vector.memset(pi2_sb, PI_2)

        # Warmup: absorb the one-time trig ACT-table load
        warm_sb = singles.tile([1, 1], FP32, name="warm_sb")
        nc.scalar.activation(warm_sb, out_sb[0:1, 0:1], AF.Sin)

        def bcast2(ap):
            # [128, T] -> [128, 2, T] stride-0 broadcast
            return bass.AP(ap.tensor, ap.offset, [ap.ap[0], [0, 2], ap.ap[1]])

        def zmm(pg_slot, l, m, rhs_tile, start=True, stop=True):
            """z/gx/gy-stream matmuls for (l, m) into pg_slot.
            rhs_tile: [128, 4, T] fp8 (or [128, 4, 2, T] sliced by caller)."""
            msl = slice(m * 128, (m + 1) * 128)
            if l in (1, 2):
                pairs = KPAIRS[l][m]
                for i, j in enumerate(pairs):
                    nc.tensor.matmul(
                        pg_slot, w_sb[l][:, 2 * j : 2 * j + 2, msl],
                        rhs_tile[:, 2 * j : 2 * j + 2, :],
                        start=(start and i == 0), stop=(stop and i == len(pairs) - 1),
                        perf_mode=DR,
                    )
            else:
                j = KPAIRS[l][m][0]
                nc.tensor.matmul(
                    pg_slot, wz_sb[l][:, m, :, :], rhs_tile[:, 2 * j : 2 * j + 2, :],
                    start=start, stop=stop, perf_mode=DR,
                )

        # ---------------- tile bodies ----------------
        def domain_tile(ti):
            csl = slice(ti * T, (ti + 1) * T)

            # layer 0: z0 = W0^T xa (K=2, bf16)
            v8 = acts.tile([128, 4, T], FP8, name=f"v_0_{ti}", tag="v")
            c0t8 = acts.tile([128, 4, T], FP8, name=f"c0_{ti}", tag="c0")
            for m in range(4):
                pg0 = ppz.tile([128, T], FP32, name=f"pg0_{ti}_{m}", tag="pgz")
                nc.tensor.matmul(
                    pg0, w0_sb[:, m * 128 : (m + 1) * 128], xa_sb[:, csl],
                    start=True, stop=True,
                )
                nc.scalar.activation(v8[:, m, :], pg0, AF.Sin)
                nc.scalar.activation(c0t8[:, m, :], pg0, AF.Sin, bias=pi2_sb[:])
            g8 = None
            mq8 = None

            # hidden layers
            for l in range(1, 5):
                v8n = acts.tile([128, 4, T], FP8, name=f"v_{l}_{ti}", tag="v")
                g8n = (acts.tile([128, 4, 2, T], FP8, name=f"g_{l}_{ti}", tag="g")
                       if l < 4 else None)
                mq8n = acts.tile([128, 4, 2, T], FP8, name=f"mq_{l}_{ti}", tag="mq")
                for m in range(4):
                    pgz = ppz.tile([128, T], FP32, name=f"pgz_{l}_{ti}_{m}", tag="pgz")
                    pgx = ppx.tile([128, 2, T], FP32, name=f"pgx_{l}_{ti}_{m}", tag="pgx")
                    pgt = ppt.tile([128, T], FP32, name=f"pgt_{l}_{ti}_{m}", tag="pgt")
                    msl = slice(m * 128, (m + 1) * 128)
                    # streams: slot0=z, slot1=gx, slot2=gy, slot3=t
                    if l == 1:
                        zmm(pgz, 1, m, v8)
                        for j in (0, 1):
                            for s, wsrc in ((1, wf_sb["x"]), (2, wf_sb["y"])):
                                nc.tensor.matmul(
                                    pgx[:, s - 1, :], wsrc[:, 2 * j : 2 * j + 2, msl],
                                    c0t8[:, 2 * j : 2 * j + 2, :],
                                    start=(j == 0), stop=(j == 1), perf_mode=DR,
                                )
                            nc.tensor.matmul(
                                pgt, wf_sb["q"][:, 2 * j : 2 * j + 2, msl],
                                v8[:, 2 * j : 2 * j + 2, :],
                                start=(j == 0), stop=(j == 1), perf_mode=DR,
                            )
                    else:
                        zmm(pgz, l, m, v8)
                        for s, xy in ((1, 0), (2, 1)):
                            if l in (1, 2):
                                pairs = KPAIRS[l][m]
                                for i, j in enumerate(pairs):
                                    nc.tensor.matmul(
                                        pgx[:, s - 1, :], w_sb[l][:, 2 * j : 2 * j + 2, msl],
                                        g8[:, 2 * j : 2 * j + 2, xy, :],
                                        start=(i == 0), stop=(i == len(pairs) - 1),
                                        perf_mode=DR,
                                    )
                            else:
                                j = KPAIRS[l][m][0]
                                nc.tensor.matmul(
                                    pgx[:, s - 1, :], wz_sb[l][:, m, :, :],
                                    g8[:, 2 * j : 2 * j + 2, xy, :],
                                    start=True, stop=True, perf_mode=DR,
                                )
                        ks = KSETS[l][m]
                        for ki, k in enumerate(ks):
                            nc.tensor.matmul(
                                pgt, wt_sb[:, WT_IDX[(l, m, ki)], :, :],
                                mq8[:, k, :, :],
                                start=(ki == 0), stop=(ki == len(ks) - 1),
                                perf_mode=DR,
                            )
                    # ---- elementwise for (l, m) ----
                    # emission order: PSUM-only consumers first (sq), then the
                    # ACT chain, then cross-engine dependents.
                    pz = pgz
                    pxy = pgx
                    pt = pgt
                    nc.scalar.activation(v8n[:, m, :], pz, AF.Sin)
                    sq = ew.tile([128, 2, T], BF16, name=f"sq_{l}_{ti}_{m}", tag="sq")
                    nc.scalar.activation(sq, pxy, AF.Square)
                    ctp = ew.tile([128, T], BF16, name=f"ct_{l}_{ti}_{m}", tag="ct")
                    ct_eng = cfg["ct"][l]
                    if ct_eng == "A":
                        # true cos(z) in bf16
                        nc.scalar.activation(ctp, pz, AF.Sin, bias=pi2_sb[:])
                        ct_is_cos = True
                    elif ct_eng == "C":
                        # split chain: v^2 on Pool (TT), 1 - v^2/2 via DVE TSP (4x)
                        s2v = ew.tile([128, T], BF16, name=f"s2_{l}_{ti}_{m}", tag="s2")
                        nc.gpsimd.tensor_tensor(
                            s2v, v8n[:, m, :], v8n[:, m, :], op=ALU.mult)
                        nc.vector.tensor_scalar(
                            ctp, s2v, -0.5, 1.0, op0=ALU.mult, op1=ALU.add)
                        ct_is_cos = True
                    else:
                        # ct' = -v^2/2 (cos = 1 + ct'); STT is DVE-only ISA
                        nc.vector.scalar_tensor_tensor(
                            ctp, v8n[:, m, :], -0.5, v8n[:, m, :],
                            op0=ALU.mult, op1=ALU.mult)
                        ct_is_cos = False

                    def ctmul(out, src, eng_key, two=False):
                        # PSUM operand -> DVE only; STT folds the +1
                        ct_ap = bcast2(ctp) if two else ctp
                        op0 = ALU.mult if ct_is_cos else ALU.add
                        nc.vector.scalar_tensor_tensor(
                            out, ct_ap, 1.0, src, op0=op0, op1=ALU.mult)

                    ctmul(mq8n[:, m, 0, :], pt, cfg["m1"][l])
                    if g8n is not None:
                        ctmul(g8n[:, m, :, :], pxy, cfg["g8"][l], two=True)
                    r2 = ew.tile([128, T], BF16, name=f"r2_{l}_{ti}_{m}", tag="r2")
                    eng = nc.vector if cfg["r2"][l] == "D" else nc.gpsimd
                    eng.tensor_tensor(r2, sq[:, 0, :], sq[:, 1, :], op=ALU.add)
                    eng = nc.vector if cfg["q8"][l] == "D" else nc.gpsimd
                    eng.tensor_tensor(
                        mq8n[:, m, 1, :], v8n[:, m, :], r2, op=ALU.mult)
                v8, g8, mq8 = v8n, g8n, mq8n

            return v8, mq8

        def domain_final(ti, v8, mq8):
            csl = slice(ti * T, (ti + 1) * T)
            # final layer: E = (-W5)^T(m1+q) + (k0^2 W5)^T v + (f + k0^2 b5)
            pgE = ppt.tile([128, T], FP32, name=f"pe_{ti}", tag="pgt")
            e = pgE[0:1, :]
            idx = 0
            for k in range(4):
                for s in (0, 1):
                    nc.tensor.matmul(e, w5t_sb[:, k, s, :], mq8[:, k, s, :],
                                     start=(idx == 0), stop=False)
                    idx += 1
                nc.tensor.matmul(e, w5v_sb[:, k, :], v8[:, k, :],
                                 start=False, stop=False)
            nc.tensor.matmul(e, one_sb, fb_sb[0:1, csl], start=False, stop=True)
            scr = ew.tile([1, T], FP32, name=f"scr_{ti}", tag="scr")
            nc.scalar.activation(scr, e, AF.Square,
                                 accum_out=out_sb[0:1, ti : ti + 1])

        bnd_vb = {}

        def boundary_step(ti, step):
            csl = slice(ti * T, (ti + 1) * T)
            if step == 0:
                vb8 = acts.tile([128, 4, T], FP8, name=f"vb_0_{ti}", tag="vb")
                for m in range(4):
                    pgb = ppz.tile([128, T], FP32, name=f"bpg0_{ti}_{m}", tag="pgz")
                    nc.tensor.matmul(
                        pgb, w0_sb[:, m * 128 : (m + 1) * 128], xb_sb[:, csl],
                        start=True, stop=True,
                    )
                    nc.scalar.activation(vb8[:, m, :], pgb, AF.Sin)
                bnd_vb[ti] = vb8
            elif step <= 4:
                l = step
                vb8 = bnd_vb[ti]
                vb8n = acts.tile([128, 4, T], FP8, name=f"vb_{l}_{ti}", tag="vb")
                for m in range(4):
                    pgn = ppz.tile([128, T], FP32, name=f"bpg_{l}_{ti}_{m}", tag="pgz")
                    zmm(pgn, l, m, vb8)
                    nc.scalar.activation(vb8n[:, m, :], pgn, AF.Sin)
                bnd_vb[ti] = vb8n
            else:
                vb8 = bnd_vb.pop(ti)
                pgE = ppt.tile([128, T], FP32, name=f"bpe_{ti}", tag="pgt")
                e = pgE[0:1, :]
                for k in range(4):
                    nc.tensor.matmul(e, w5b_sb[:, k, :], vb8[:, k, :],
                                     start=(k == 0), stop=False)
                nc.tensor.matmul(e, one_sb, bb_sb[0:1, csl], start=False, stop=True)
                scr = ew.tile([1, T], FP32, name=f"bscr_{ti}", tag="scr")
                nc.scalar.activation(scr, e, AF.Square,
                                     accum_out=out_sb[0:1, 16 + ti : 17 + ti])

        # spread boundary work: 6 steps per boundary tile woven through the
        # domain schedule (1-2 steps after each domain tile)
        bsteps = [(bi, s) for s in range(6) for bi in range(ntb)]
        total = len(bsteps)
        done = 0
        pending_final = None
        for ti in range(ntd):
            state = domain_tile(ti)
            if pending_final is not None:
                domain_final(ti - 1, *pending_final)
            pending_final = state
            want = (ti + 1) * total // ntd
            while done < want:
                boundary_step(*bsteps[done])
                done += 1
        domain_final(ntd - 1, *pending_final)
        while done < total:
            boundary_step(*bsteps[done])
            done += 1

        nc.sync.dma_start(out=out_d[:], in_=out_sb)
    nc.compile()
    return nc


def _masks():
    layers = [2, 512, 256, 128, 64, 32, 1]
    width = [2, 512, 512, 512, 512, 512, 1]
    masks = {}
    for l in range(2, 5):
        nb_ = 2 ** (l - 1)
        bs1 = width[l] // nb_
        bs2 = 2 * layers[l + 1]
        m = np.zeros((512, 512), np.float32)
        for i in range(nb_):
            m[i * bs1 : (i + 1) * bs1, i * bs2 : (i + 1) * bs2] = 1.0
        masks[l] = m
    return masks


def _chunked(w):
    # [512, N] -> [128, 4, N] with out[p, kt, j] = w[kt*128 + p, j]
    n = w.shape[1]
    return np.ascontiguousarray(w.reshape(4, 128, n).transpose(1, 0, 2))


def host_prep(inputs, ntd=NTD, ntb=NTB):
    X = np.asarray(inputs["X_train"], np.float32)
    W = [np.asarray(inputs[f"W{i}"], np.float32) for i in range(6)]
    b = [np.asarray(inputs[f"b{i}"], np.float32) for i in range(6)]
    assert all(np.all(bi == 0.0) for bi in b[:5]), "nonzero hidden biases unsupported"
    for l, m in _masks().items():
        W[l] = W[l] * m
    Wc = {l: _chunked(W[l]) for l in (1, 2, 3, 4)}

    shared = {"w0": W[0].astype(bf16)}
    for l in (1, 2):
        shared[f"w{l}"] = Wc[l].astype(f8)

    zx0 = 2.0 * W[0][0, :]
    zy0 = 2.0 * W[0][1, :]
    c2 = zx0 ** 2 + zy0 ** 2
    shared["w1x"] = _chunked(zx0[:, None] * W[1]).astype(f8)
    shared["w1y"] = _chunked(zy0[:, None] * W[1]).astype(f8)
    shared["w1q"] = _chunked(c2[:, None] * W[1]).astype(f8)

    for l in (3, 4):
        wz = np.zeros((128, 4, 2, 128), np.float32)
        for m in range(4):
            k = KSETS[l][m][0]
            wz[:, m, k % 2, :] = Wc[l][:, k, m * 128 : (m + 1) * 128]
        shared[f"wz{l}"] = wz.astype(f8)

    wt = np.zeros((128, NWT, 2, 128), np.float32)
    for (l, m, ki), row in WT_IDX.items():
        k = KSETS[l][m][ki]
        blk = Wc[l][:, k, m * 128 : (m + 1) * 128]
        wt[:, row, 0, :] = blk
        wt[:, row, 1, :] = blk
    shared["wt"] = wt.astype(f8)

    w5c = _chunked(W[5])  # [128, 4, 1]
    w5t = np.zeros((128, 4, 2, 1), np.float32)
    w5t[:, :, 0, :] = -w5c
    w5t[:, :, 1, :] = -w5c
    shared["w5t"] = w5t.astype(f8)
    shared["w5v"] = (K0SQ * w5c).astype(f8)
    shared["w5b"] = w5c.astype(f8)

    b5 = float(b[5][0, 0])
    td, tb = ntd * T, ntb * T
    per_core = []
    for c in range(NCORES):
        Xd = X[c * TDOM : c * TDOM + td]
        Xb = X[ND + c * TBND : ND + c * TBND + tb]
        xa = np.ascontiguousarray((2.0 * Xd - 1.0).T).astype(bf16)
        xbt = np.ascontiguousarray((2.0 * Xb - 1.0).T).astype(bf16)
        fv = (K0SQ * np.sin(K0 * Xd[:, 0].astype(np.float64))
              * np.sin(K0 * Xd[:, 1].astype(np.float64)))
        fb = (fv + K0SQ * b5).astype(np.float32).reshape(1, td)
        bb = np.full((1, tb), b5, np.float32)
        per_core.append({"xa": xa, "xb": xbt, "fb": fb, "bb": bb})
    shared["onec"] = np.ones((1, 1), np.float32)
    return shared, per_core


_CACHE = {}


def _run(inputs, trace=False):
    key = "nc"
    if key not in _CACHE:
        _CACHE[key] = build_nc()
    nc = _CACHE[key]
    shared, per_core = host_prep(inputs)
    in_maps = [dict(shared, **pc) for pc in per_core]
    res = run_bass_kernel_spmd(nc, in_maps, core_ids=list(range(NCORES)), trace=trace)
    outs = [r["out"] for r in res.results]
    se = sum(float(o[0, :NTD].sum()) for o in outs)
    sb = sum(float(o[0, 16 : 16 + NTB].sum()) for o in outs)
    loss = se / ND + 100.0 * sb / NB
    return np.float32(loss), res


def kernel(**inputs):
    loss, _ = _run(inputs, trace=False)
    return np.asarray(loss)


# revision 23
# speedup vs baseline: 1.0004x; 1.0004x over previous
"""Trainium2 Bass kernel for the BsPINN Helmholtz loss — fp8 DoubleRow edition.

Math identical to v1 (forward-Laplacian with streams v, gx, gy, t=m1+q), but:
  - all hidden matmuls use fp8e4 inputs with MatmulPerfMode.DoubleRow, pairing
    k-chunks (dense layers), (W,0) half-pairs (block-diagonal single-chunk
    layers), and (m1,q) stream pairs (the t-stream add is absorbed by the pair)
  - cos(z) comes from 1 - sin(z)^2/2 ... exactly: ct' = -v^2/2, and the (+1)
    is folded into the scalar_tensor_tensor ops (ct'+1)*x
  - f/bias adds ride float32r K=1 matmuls (1 cycle/row at free>=256)
  - elementwise split by ISA constraints: sin/Square of PSUM on ACT (only
    engine with single-read PSUM math), PSUM multiplies on DVE (Pool cannot
    access PSUM; no engine may read PSUM twice in one instruction), and
    SBUF-only ops (q8, r2) on Pool; boundary tiles are woven through the
    domain schedule in per-layer steps to fill engine gaps

Sharding: data-parallel, 8 cores x (8192 domain + 2048 boundary) points.
"""

import numpy as np
import ml_dtypes

import concourse.bass as bass
import concourse.bacc as bacc_mod
import concourse.mybir as mybir
import concourse.tile as tile
from concourse.bass_utils import run_bass_kernel_spmd

bf16 = ml_dtypes.bfloat16
f8 = ml_dtypes.float8_e4m3
FP32 = mybir.dt.float32
F32R = mybir.dt.float32r
BF16 = mybir.dt.bfloat16
FP8 = mybir.dt.float8e4
AF = mybir.ActivationFunctionType
ALU = mybir.AluOpType
DR = mybir.MatmulPerfMode.DoubleRow

NCORES = 8
ND, NB = 65536, 16384
TDOM, TBND = ND // NCORES, NB // NCORES  # 8192, 2048
T = 512
NTD, NTB = TDOM // T, TBND // T          # 16, 4
K0 = 8.0
K0SQ = K0 * K0
PI_2 = float(np.pi / 2)

# k-chunk sets per (hidden layer, output m-chunk) from the block-diagonal masks
KSETS = {
    1: [[0, 1, 2, 3]] * 4,
    2: [[0, 1], [0, 1], [2, 3], [2, 3]],
    3: [[0], [1], [2], [3]],
    4: [[0], [1], [2], [3]],
}
# k-pair indices (pair j covers chunks 2j, 2j+1) for the z/gx/gy streams
KPAIRS = {
    1: [[0, 1]] * 4,
    2: [[0], [0], [1], [1]],
    3: [[0], [0], [1], [1]],   # half-pair via wz (slot m%2 live)
    4: [[0], [0], [1], [1]],
}
# index into the wt (m1,q)-pair weight tensor: (l, m, ki) -> row
WT_IDX = {}
_row = 0
for _l in (2, 3, 4):
    for _m in range(4):
        for _ki, _k in enumerate(KSETS[_l][_m]):
            WT_IDX[(_l, _m, _ki)] = _row
            _row += 1
NWT = _row  # 16


# engine assignment config: per hidden layer l=1..4, engines for each op.
# 'A' = ACT (scalar), 'D' = DVE (vector), 'P' = Pool (gpsimd)
# ct: cos carrier ('A' = Sin(z+pi/2) direct cos bf16; 'D'/'P' = STT 1-v8^2/2)
# Constraints: Square/sin of PSUM: ACT only (single PSUM read per inst,
# Pool cannot access PSUM). g8/m1 (PSUM muls): DVE only. SBUF-only ops
# (ct-from-v8, r2, q8) can go to Pool.
DEFAULT_CFG = {
    "ct": {1: "D", 2: "D", 3: "D", 4: "D"},
    "sq": {1: "A", 2: "A", 3: "A", 4: "A"},
    "g8": {1: "D", 2: "D", 3: "D"},
    "m1": {1: "D", 2: "D", 3: "D", 4: "D"},
    "q8": {1: "P", 2: "P", 3: "P", 4: "P"},
    "r2": {1: "D", 2: "P", 3: "P", 4: "P"},
}


def build_nc(ntd=NTD, ntb=NTB, cfg=None):
    from contextlib import ExitStack
    cfg = cfg or DEFAULT_CFG

    td, tb = ntd * T, ntb * T
    nc = bacc_mod.Bacc("TRN2", target_bir_lowering=False)

    xa_d = nc.dram_tensor("xa", [2, td], BF16, kind="ExternalInput")
    xb_d = nc.dram_tensor("xb", [2, tb], BF16, kind="ExternalInput")
    fb_d = nc.dram_tensor("fb", [1, td], F32R, kind="ExternalInput")
    bb_d = nc.dram_tensor("bb", [1, tb], F32R, kind="ExternalInput")
    w0_d = nc.dram_tensor("w0", [2, 512], BF16, kind="ExternalInput")
    w_d = {
        l: nc.dram_tensor(f"w{l}", [128, 4, 512], FP8, kind="ExternalInput")
        for l in (1, 2)
    }
    wf_d = {
        s: nc.dram_tensor(f"w1{s}", [128, 4, 512], FP8, kind="ExternalInput")
        for s in ("x", "y", "q")
    }
    wz_d = {
        l: nc.dram_tensor(f"wz{l}", [128, 4, 2, 128], FP8, kind="ExternalInput")
        for l in (3, 4)
    }
    wt_d = nc.dram_tensor("wt", [128, NWT, 2, 128], FP8, kind="ExternalInput")
    w5t_d = nc.dram_tensor("w5t", [128, 4, 2, 1], FP8, kind="ExternalInput")
    w5v_d = nc.dram_tensor("w5v", [128, 4, 1], FP8, kind="ExternalInput")
    w5b_d = nc.dram_tensor("w5b", [128, 4, 1], FP8, kind="ExternalInput")
    one_d = nc.dram_tensor("onec", [1, 1], F32R, kind="ExternalInput")
    out_d = nc.dram_tensor("out", [1, 32], FP32, kind="ExternalOutput")

    with tile.TileContext(nc) as tc, ExitStack() as ctx:
        singles = ctx.enter_context(tc.tile_pool(name="singles", bufs=1))
        acts = ctx.enter_context(tc.tile_pool(name="acts", bufs=6))
        ew = ctx.enter_context(tc.tile_pool(name="ew", bufs=6))
        ppz = ctx.enter_context(tc.tile_pool(name="ppz", bufs=2, space="PSUM"))
        ppx = ctx.enter_context(tc.tile_pool(name="ppx", bufs=2, space="PSUM"))
        ppt = ctx.enter_context(tc.tile_pool(name="ppt", bufs=2, space="PSUM"))

        def ld(name, shape, dt, src):
            t_ = singles.tile(shape, dt, name=name, tag=name)
            nc.sync.dma_start(out=t_, in_=src[:])
            return t_

        w0_sb = ld("w0_sb", [2, 512], BF16, w0_d)
        xa_sb = singles.tile([2, td], BF16, name="xa_sb", tag="xa_sb")
        hd = min(2 * T, td)
        nc.sync.dma_start(out=xa_sb[:, 0:hd], in_=xa_d[:, 0:hd])
        if hd < td:
            nc.sync.dma_start(out=xa_sb[:, hd:td], in_=xa_d[:, hd:td])
        w_sb = {l: ld(f"w{l}_sb", [128, 4, 512], FP8, w_d[l]) for l in (1, 2)}
        wf_sb = {s: ld(f"w1{s}_sb", [128, 4, 512], FP8, wf_d[s]) for s in ("x", "y", "q")}
        wz_sb = {l: ld(f"wz{l}_sb", [128, 4, 2, 128], FP8, wz_d[l]) for l in (3, 4)}
        wt_sb = ld("wt_sb", [128, NWT, 2, 128], FP8, wt_d)
        xb_sb = ld("xb_sb", [2, tb], BF16, xb_d)
        fb_sb = ld("fb_sb", [1, td], F32R, fb_d)
        bb_sb = ld("bb_sb", [1, tb], F32R, bb_d)
        w5t_sb = ld("w5t_sb", [128, 4, 2, 1], FP8, w5t_d)
        w5v_sb = ld("w5v_sb", [128, 4, 1], FP8, w5v_d)
        w5b_sb = ld("w5b_sb", [128, 4, 1], FP8, w5b_d)

        one_sb = ld("one_sb", [1, 1], F32R, one_d)
        out_sb = singles.tile([1, 32], FP32, name="out_sb")
        nc.vector.memset(out_sb, 0.0)

        pi2_sb = singles.tile([128, 1], FP32, name="pi2_sb")
        nc.vector.memset(pi2_sb, PI_2)

        # Warmup: absorb the one-time trig ACT-table load
        warm_sb = singles.tile([1, 1], FP32, name="warm_sb")
        nc.scalar.activation(warm_sb, out_sb[0:1, 0:1], AF.Sin)

        def bcast2(ap):
            # [128, T] -> [128, 2, T] stride-0 broadcast
            return bass.AP(ap.tensor, ap.offset, [ap.ap[0], [0, 2], ap.ap[1]])

        def zmm(pg_slot, l, m, rhs_tile, start=True, stop=True):
            """z/gx/gy-stream matmuls for (l, m) into pg_slot.
            rhs_tile: [128, 4, T] fp8 (or [128, 4, 2, T] sliced by caller)."""
            msl = slice(m * 128, (m + 1) * 128)
            if l in (1, 2):
                pairs = KPAIRS[l][m]
                for i, j in enumerate(pairs):
                    nc.tensor.matmul(
                        pg_slot, w_sb[l][:, 2 * j : 2 * j + 2, msl],
                        rhs_tile[:, 2 * j : 2 * j + 2, :],
                        start=(start and i == 0), stop=(stop and i == len(pairs) - 1),
                        perf_mode=DR,
                    )
            else:
                j = KPAIRS[l][m][0]
                nc.tensor.matmul(
                    pg_slot, wz_sb[l][:, m, :, :], rhs_tile[:, 2 * j : 2 * j + 2, :],
                    start=start, stop=stop, perf_mode=DR,
                )

        # ---------------- tile bodies ----------------
        def domain_tile(ti):
            csl = slice(ti * T, (ti + 1) * T)

            # layer 0: z0 = W0^T xa (K=2, bf16)
            v8 = acts.tile([128, 4, T], FP8, name=f"v_0_{ti}", tag="v")
            c0t8 = acts.tile([128, 4, T], FP8, name=f"c0_{ti}", tag="c0")
            for m in range(4):
                pg0 = ppz.tile([128, T], FP32, name=f"pg0_{ti}_{m}", tag="pgz")
                nc.tensor.matmul(
                    pg0, w0_sb[:, m * 128 : (m + 1) * 128], xa_sb[:, csl],
                    start=True, stop=True,
                )
                nc.scalar.activation(v8[:, m, :], pg0, AF.Sin)
                nc.scalar.activation(c0t8[:, m, :], pg0, AF.Sin, bias=pi2_sb[:])
            g8 = None
            mq8 = None

            # hidden layers
            for l in range(1, 5):
                v8n = acts.tile([128, 4, T], FP8, name=f"v_{l}_{ti}", tag="v")
                g8n = (acts.tile([128, 4, 2, T], FP8, name=f"g_{l}_{ti}", tag="g")
                       if l < 4 else None)
                mq8n = acts.tile([128, 4, 2, T], FP8, name=f"mq_{l}_{ti}", tag="mq")
                for m in range(4):
                    pgz = ppz.tile([128, T], FP32, name=f"pgz_{l}_{ti}_{m}", tag="pgz")
                    pgx = ppx.tile([128, 2, T], FP32, name=f"pgx_{l}_{ti}_{m}", tag="pgx")
                    pgt = ppt.tile([128, T], FP32, name=f"pgt_{l}_{ti}_{m}", tag="pgt")
                    msl = slice(m * 128, (m + 1) * 128)
                    # streams: slot0=z, slot1=gx, slot2=gy, slot3=t
                    if l == 1:
                        zmm(pgz, 1, m, v8)
                        for j in (0, 1):
                            for s, wsrc in ((1, wf_sb["x"]), (2, wf_sb["y"])):
                                nc.tensor.matmul(
                                    pgx[:, s - 1, :], wsrc[:, 2 * j : 2 * j + 2, msl],
                                    c0t8[:, 2 * j : 2 * j + 2, :],
                                    start=(j == 0), stop=(j == 1), perf_mode=DR,
                                )
                            nc.tensor.matmul(
                                pgt, wf_sb["q"][:, 2 * j : 2 * j + 2, msl],
                                v8[:, 2 * j : 2 * j + 2, :],
                                start=(j == 0), stop=(j == 1), perf_mode=DR,
                            )
                    else:
                        zmm(pgz, l, m, v8)
                        for s, xy in ((1, 0), (2, 1)):
                            if l in (1, 2):
                                pairs = KPAIRS[l][m]
                                for i, j in enumerate(pairs):
                                    nc.tensor.matmul(
                                        pgx[:, s - 1, :], w_sb[l][:, 2 * j : 2 * j + 2, msl],
                                        g8[:, 2 * j : 2 * j + 2, xy, :],
                                        start=(i == 0), stop=(i == len(pairs) - 1),
                                        perf_mode=DR,
                                    )
                            else:
                                j = KPAIRS[l][m][0]
                                nc.tensor.matmul(
                                    pgx[:, s - 1, :], wz_sb[l][:, m, :, :],
                                    g8[:, 2 * j : 2 * j + 2, xy, :],
                                    start=True, stop=True, perf_mode=DR,
                                )
                        ks = KSETS[l][m]
                        for ki, k in enumerate(ks):
                            nc.tensor.matmul(
                                pgt, wt_sb[:, WT_IDX[(l, m, ki)], :, :],
                                mq8[:, k, :, :],
                                start=(ki == 0), stop=(ki == len(ks) - 1),
                                perf_mode=DR,
                            )
                    # ---- elementwise for (l, m) ----
                    # emission order: PSUM-only consumers first (sq), then the
                    # ACT chain, then cross-engine dependents.
                    pz = pgz
                    pxy = pgx
                    pt = pgt
                    nc.scalar.activation(v8n[:, m, :], pz, AF.Sin)
                    sq = ew.tile([128, 2, T], BF16, name=f"sq_{l}_{ti}_{m}", tag="sq")
                    nc.scalar.activation(sq, pxy, AF.Square)
                    ctp = ew.tile([128, T], BF16, name=f"ct_{l}_{ti}_{m}", tag="ct")
                    ct_eng = cfg["ct"][l]
                    if ct_eng == "A":
                        # true cos(z) in bf16
                        nc.scalar.activation(ctp, pz, AF.Sin, bias=pi2_sb[:])
                        ct_is_cos = True
                    elif ct_eng == "C":
                        # split chain: v^2 on Pool (TT), 1 - v^2/2 via DVE TSP (4x)
                        s2v = ew.tile([128, T], BF16, name=f"s2_{l}_{ti}_{m}", tag="s2")
                        nc.gpsimd.tensor_tensor(
                            s2v, v8n[:, m, :], v8n[:, m, :], op=ALU.mult)
                        nc.vector.tensor_scalar(
                            ctp, s2v, -0.5, 1.0, op0=ALU.mult, op1=ALU.add)
                        ct_is_cos = True
                    else:
                        # ct' = -v^2/2 (cos = 1 + ct'); STT is DVE-only ISA
                        nc.vector.scalar_tensor_tensor(
                            ctp, v8n[:, m, :], -0.5, v8n[:, m, :],
                            op0=ALU.mult, op1=ALU.mult)
                        ct_is_cos = False

                    def ctmul(out, src, eng_key, two=False):
                        # PSUM operand -> DVE only; STT folds the +1
                        ct_ap = bcast2(ctp) if two else ctp
                        op0 = ALU.mult if ct_is_cos else ALU.add
                        nc.vector.scalar_tensor_tensor(
                            out, ct_ap, 1.0, src, op0=op0, op1=ALU.mult)

                    ctmul(mq8n[:, m, 0, :], pt, cfg["m1"][l])
                    if g8n is not None:
                        ctmul(g8n[:, m, :, :], pxy, cfg["g8"][l], two=True)
                    r2 = ew.tile([128, T], BF16, name=f"r2_{l}_{ti}_{m}", tag="r2")
                    eng = nc.vector if cfg["r2"][l] == "D" else nc.gpsimd
                    eng.tensor_tensor(r2, sq[:, 0, :], sq[:, 1, :], op=ALU.add)
                    eng = nc.vector if cfg["q8"][l] == "D" else nc.gpsimd
                    eng.tensor_tensor(
                        mq8n[:, m, 1, :], v8n[:, m, :], r2, op=ALU.mult)
                v8, g8, mq8 = v8n, g8n, mq8n

            return v8, mq8

        def domain_final(ti, v8, mq8):
            csl = slice(ti * T, (ti + 1) * T)
            # final layer: E = (-W5)^T(m1+q) + (k0^2 W5)^T v + (f + k0^2 b5)
            pgE = ppt.tile([128, T], FP32, name=f"pe_{ti}", tag="pgt")
            e = pgE[0:1, :]
            idx = 0
            for k in range(4):
                for s in (0, 1):
                    nc.tensor.matmul(e, w5t_sb[:, k, s, :], mq8[:, k, s, :],
                                     start=(idx == 0), stop=False)
                    idx += 1
                nc.tensor.matmul(e, w5v_sb[:, k, :], v8[:, k, :],
                                 start=False, stop=False)
            nc.tensor.matmul(e, one_sb, fb_sb[0:1, csl], start=False, stop=True)
            scr = ew.tile([1, T], FP32, name=f"scr_{ti}", tag="scr")
            nc.scalar.activation(scr, e, AF.Square,
                                 accum_out=out_sb[0:1, ti : ti + 1])

        bnd_vb = {}

        def boundary_step(ti, step):
            csl = slice(ti * T, (ti + 1) * T)
            if step == 0:
                vb8 = acts.tile([128, 4, T], FP8, name=f"vb_0_{ti}", tag="vb")
                for m in range(4):
                    pgb = ppz.tile([128, T], FP32, name=f"bpg0_{ti}_{m}", tag="pgz")
                    nc.tensor.matmul(
                        pgb, w0_sb[:, m * 128 : (m + 1) * 128], xb_sb[:, csl],
                        start=True, stop=True,
                    )
                    nc.scalar.activation(vb8[:, m, :], pgb, AF.Sin)
                bnd_vb[ti] = vb8
            elif step <= 4:
                l = step
                vb8 = bnd_vb[ti]
                vb8n = acts.tile([128, 4, T], FP8, name=f"vb_{l}_{ti}", tag="vb")
                for m in range(4):
                    pgn = ppz.tile([128, T], FP32, name=f"bpg_{l}_{ti}_{m}", tag="pgz")
                    zmm(pgn, l, m, vb8)
                    nc.scalar.activation(vb8n[:, m, :], pgn, AF.Sin)
                bnd_vb[ti] = vb8n
            else:
                vb8 = bnd_vb.pop(ti)
                pgE = ppt.tile([128, T], FP32, name=f"bpe_{ti}", tag="pgt")
                e = pgE[0:1, :]
                for k in range(4):
                    nc.tensor.matmul(e, w5b_sb[:, k, :], vb8[:, k, :],
                                     start=(k == 0), stop=False)
                nc.tensor.matmul(e, one_sb, bb_sb[0:1, csl], start=False, stop=True)
                scr = ew.tile([1, T], FP32, name=f"bscr_{ti}", tag="scr")
                nc.scalar.activation(scr, e, AF.Square,
                                     accum_out=out_sb[0:1, 16 + ti : 17 + ti])

        # spread boundary work: 6 steps per boundary tile woven through the
        # domain schedule (1-2 steps after each domain tile)
        bsteps = [(bi, s) for s in range(6) for bi in range(ntb)]
        total = len(bsteps)
        done = 0
        pending_final = None
        for ti in range(ntd):
            state = domain_tile(ti)
            if pending_final is not None:
                domain_final(ti - 1, *pending_final)
            pending_final = state
            want = (ti + 1) * total // ntd
            while done < want:
                boundary_step(*bsteps[done])
                done += 1
        domain_final(ntd - 1, *pending_final)
        while done < total:
            boundary_step(*bsteps[done])
            done += 1

        nc.sync.dma_start(out=out_d[:], in_=out_sb)
    nc.compile()
    return nc


def _masks():
    layers = [2, 512, 256, 128, 64, 32, 1]
    width = [2, 512, 512, 512, 512, 512, 1]
    masks = {}
    for l in range(2, 5):
        nb_ = 2 ** (l - 1)
        bs1 = width[l] // nb_
        bs2 = 2 * layers[l + 1]
        m = np.zeros((512, 512), np.float32)
        for i in range(nb_):
            m[i * bs1 : (i + 1) * bs1, i * bs2 : (i + 1) * bs2] = 1.0
        masks[l] = m
    return masks


def _chunked(w):
    # [512, N] -> [128, 4, N] with out[p, kt, j] = w[kt*128 + p, j]
    n = w.shape[1]
    return np.ascontiguousarray(w.reshape(4, 128, n).transpose(1, 0, 2))


def host_prep(inputs, ntd=NTD, ntb=NTB):
    X = np.asarray(inputs["X_train"], np.float32)
    W = [np.asarray(inputs[f"W{i}"], np.float32) for i in range(6)]
    b = [np.asarray(inputs[f"b{i}"], np.float32) for i in range(6)]
    assert all(np.all(bi == 0.0) for bi in b[:5]), "nonzero hidden biases unsupported"
    for l, m in _masks().items():
        W[l] = W[l] * m
    Wc = {l: _chunked(W[l]) for l in (1, 2, 3, 4)}

    shared = {"w0": W[0].astype(bf16)}
    for l in (1, 2):
        shared[f"w{l}"] = Wc[l].astype(f8)

    zx0 = 2.0 * W[0][0, :]
    zy0 = 2.0 * W[0][1, :]
    c2 = zx0 ** 2 + zy0 ** 2
    shared["w1x"] = _chunked(zx0[:, None] * W[1]).astype(f8)
    shared["w1y"] = _chunked(zy0[:, None] * W[1]).astype(f8)
    shared["w1q"] = _chunked(c2[:, None] * W[1]).astype(f8)

    for l in (3, 4):
        wz = np.zeros((128, 4, 2, 128), np.float32)
        for m in range(4):
            k = KSETS[l][m][0]
            wz[:, m, k % 2, :] = Wc[l][:, k, m * 128 : (m + 1) * 128]
        shared[f"wz{l}"] = wz.astype(f8)

    wt = np.zeros((128, NWT, 2, 128), np.float32)
    for (l, m, ki), row in WT_IDX.items():
        k = KSETS[l][m][ki]
        blk = Wc[l][:, k, m * 128 : (m + 1) * 128]
        wt[:, row, 0, :] = blk
        wt[:, row, 1, :] = blk
    shared["wt"] = wt.astype(f8)

    w5c = _chunked(W[5])  # [128, 4, 1]
    w5t = np.zeros((128, 4, 2, 1), np.float32)
    w5t[:, :, 0, :] = -w5c
    w5t[:, :, 1, :] = -w5c
    shared["w5t"] = w5t.astype(f8)
    shared["w5v"] = (K0SQ * w5c).astype(f8)
    shared["w5b"] = w5c.astype(f8)

    b5 = float(b[5][0, 0])
    td, tb = ntd * T, ntb * T
    per_core = []
    for c in range(NCORES):
        Xd = X[c * TDOM : c * TDOM + td]
        Xb = X[ND + c * TBND : ND + c * TBND + tb]
        xa = np.ascontiguousarray((2.0 * Xd - 1.0).T).astype(bf16)
        xbt = np.ascontiguousarray((2.0 * Xb - 1.0).T).astype(bf16)
        fv = (K0SQ * np.sin(K0 * Xd[:, 0].astype(np.float64))
              * np.sin(K0 * Xd[:, 1].astype(np.float64)))
        fb = (fv + K0SQ * b5).astype(np.float32).reshape(1, td)
        bb = np.full((1, tb), b5, np.float32)
        per_core.append({"xa": xa, "xb": xbt, "fb": fb, "bb": bb})
    shared["onec"] = np.ones((1, 1), np.float32)
    return shared, per_core


_CACHE = {}


def _run(inputs, trace=False):
    key = "nc"
    if key not in _CACHE:
        _CACHE[key] = build_nc()
    nc = _CACHE[key]
    shared, per_core = host_prep(inputs)
    in_maps = [dict(shared, **pc) for pc in per_core]
    res = run_bass_kernel_spmd(nc, in_maps, core_ids=list(range(NCORES)), trace=trace)
    outs = [r["out"] for r in res.results]
    se = sum(float(o[0, :NTD].sum()) for o in outs)
    sb = sum(float(o[0, 16 : 16 + NTB].sum()) for o in outs)
    loss = se / ND + 100.0 * sb / NB
    return np.float32(loss), res


def kernel(**inputs):
    loss, _ = _run(inputs, trace=False)
    return np.asarray(loss)
